# revision 1
# baseline (speedup 1.0000x reference)
"""CrystalGraphConvNet Trainium2 Bass kernel (8-core SPMD, full I/O).

Sharding: data-parallel over atoms (12500/core, crystal-agnostic); the h
table is replicated in HBM as bf16 [8*12544, 128] (zero-padded shards) and
rebuilt each layer via AllGather. Edges are grouped per core by
(atom-range x j-window) so the neighbor gather (dma_gather int16, transpose
mode -> feature-major tiles) and the message scatter (dma_scatter_add int16
into unique (atom,m) slots of a wide buffer) both stay in int16 range.
BatchNorm batch statistics are AllReduce'd. Pooling runs redundantly on all
cores over the AllGather'd final h with host-built one-hot band matrices
(crystal boundaries baked from the input); core 0's output is returned.
"""
import sys
sys.path.insert(0, '/opt/trn_rl_repo')

import numpy as np
import ml_dtypes

import concourse.bass as bass
import concourse.mybir as mybir
import concourse.tile as tile
from concourse import bacc
from concourse.bass import ds, ts
from concourse.masks import make_identity

BF16 = mybir.dt.bfloat16
F32 = mybir.dt.float32
I16 = mybir.dt.int16
AF = mybir.ActivationFunctionType
BF = ml_dtypes.bfloat16

# ---------------- problem config (module globals; test may override) ----
N, M, N0 = 100000, 12, 2500
ORIG, C, NBR, H = 92, 64, 41, 128
NCONV = 3
EPS = 1e-5
R = 8
CH = 1536
NWIN = 4
NAR = 5


def _derived():
    APC = N // R
    SHPAD = -(-(APC + 1) // 128) * 128          # padded shard rows (>=1 zero row)
    TBLR = R * SHPAD
    WROWS = (R // NWIN) * SHPAD                 # rows per j-window
    ARN = -(--(-APC // NAR) // 128) * 128       # atoms per range (x128)
    NGRP = NAR * NWIN
    WIDE_AR = ARN * M
    WBS = WIDE_AR + 256                         # wide block stride (+pad)
    WIDE_ROWS = NAR * WBS
    TRASH16 = WIDE_AR                           # block-local trash slot
    ZROW16 = APC                                # in-window zero row (shard pad)
    NTILE = -(-APC // 128)
    assert SHPAD > APC and WIDE_AR + 255 <= 32767
    return APC, SHPAD, TBLR, WROWS, ARN, NGRP, WIDE_AR, WBS, WIDE_ROWS, TRASH16, ZROW16, NTILE


def _chunks(EG):
    out, e = [], 0
    while e < EG:
        c = min(CH, EG - e)
        out.append((e, c))
        e += c
    return out


def _i16lay(a):
    """[G, E] int16 -> [128, G, E//16] wrapped/replicated device layout."""
    G, E = a.shape
    out = np.zeros((128, G, E // 16), np.int16)
    for gi in range(G):
        out[:, gi, :] = np.tile(a[gi].reshape(E // 16, 16).T, (8, 1))
    return out


def prep_host(nbr_fea_idx, nbr_fea, crystal_atom_idx):
    APC, SHPAD, TBLR, WROWS, ARN, NGRP, WIDE_AR, WBS, WIDE_ROWS, TRASH16, ZROW16, NTILE = _derived()
    idx = np.asarray(nbr_fea_idx).astype(np.int64)
    fea = np.asarray(nbr_fea).astype(np.float32)
    cai = np.asarray(crystal_atom_idx).astype(np.int64)

    def trow(j):
        return (j // APC) * SHPAD + (j % APC)

    per_core = []
    sizes = np.zeros((R, NGRP), np.int64)
    for r in range(R):
        a0 = r * APC
        e_atom = np.repeat(np.arange(a0, a0 + APC), M)
        e_m = np.tile(np.arange(M), APC)
        e_j = idx[a0:a0 + APC].reshape(-1)
        loc = e_atom - a0
        g = (loc // ARN) * NWIN + (trow(e_j) // WROWS)
        order = np.argsort(g, kind='stable')
        groups = [order[g[order] == gi] for gi in range(NGRP)]
        for gi in range(NGRP):
            sizes[r, gi] = len(groups[gi])
        per_core.append((groups, e_atom, e_m, e_j, loc))

    EG = int(-(-sizes.max() // 256) * 256)
    EGT = EG * NGRP
    NCH = _chunks(EG)

    # scatter staging position k = c*128 + p ; c = i*12(+) ; edge mapping
    kpos = np.zeros(EG, np.int64)
    kk = 0
    for (eoff, clen) in NCH:
        half, ncols = clen // 2, clen // 128
        for col in range(ncols):
            s, t = col % 2, col // 2
            for p in range(128):
                kpos[kk] = eoff + s * half + t * 128 + p
                kk += 1

    cores = []
    for r in range(R):
        groups, e_atom, e_m, e_j, loc = per_core[r]
        nbr16 = np.full((NGRP, EG), ZROW16, np.int16)
        self16 = np.full((NGRP, EG), ZROW16, np.int16)
        sc16 = np.full((NGRP, EG), TRASH16, np.int16)
        feaT = np.zeros((NBR, EGT), np.float32)
        for gi in range(NGRP):
            sel = groups[gi]
            n = len(sel)
            ar, w = gi // NWIN, gi % NWIN
            nbr16[gi, :n] = trow(e_j[sel]) - w * WROWS
            self16[gi, :n] = loc[sel]
            sc16[gi, :n] = (loc[sel] - ar * ARN) * M + e_m[sel]
            feaT[:, gi * EG:gi * EG + n] = fea[e_atom[sel], e_m[sel]].T
        sc16 = sc16[:, kpos]
        cores.append(dict(
            nbr16=_i16lay(nbr16), self16=_i16lay(self16), sc16=_i16lay(sc16),
            feaT=feaT.astype(BF),
        ))

    # ---- pooling metadata (global, baked) ----
    counts = np.bincount(cai, minlength=N0).astype(np.int64)
    cstart = np.zeros(N0 + 1, np.int64)
    cstart[1:] = np.cumsum(counts)
    NB = -(-N0 // 128)
    bands = []
    for b in range(NB):
        bc0, bc1 = 128 * b, min(128 * b + 128, N0)
        s, e = int(cstart[bc0]), int(cstart[bc1])
        t0, t1 = s // 128, -(-e // 128) if e > s else (s // 128)
        if e == s:
            t0 = t1 = 0
        bands.append((bc0, bc1, t0, t1))
    TT = sum(t1 - t0 for (_, _, t0, t1) in bands)
    onehot = np.zeros((max(TT, 1), 128, 128), np.float32)
    w = 1.0 / np.maximum(counts, 1)
    ti = 0
    for (bc0, bc1, t0, t1) in bands:
        for t in range(t0, t1):
            arows = np.arange(128 * t, min(128 * t + 128, N))
            cc = cai[arows]
            ok = (cc >= bc0) & (cc < bc1)
            rr = np.nonzero(ok)[0]
            onehot[ti, rr, cc[rr] - bc0] = w[cc[rr]]
            ti += 1
    return cores, EG, EGT, bands, TT, onehot.astype(BF)


DEBUG_TAPS = False


def build(EG, bands, TT):
    APC, SHPAD, TBLR, WROWS, ARN, NGRP, WIDE_AR, WBS, WIDE_ROWS, TRASH16, ZROW16, NTILE = _derived()
    EGT = EG * NGRP
    NCH = _chunks(EG)
    NB = len(bands)
    HT_TILES = -(-N // 128)

    nc = bacc.Bacc("TRN2", target_bir_lowering=False, debug=False, num_devices=R)
    dtn = nc.dram_tensor
    afT = dtn("afT", [ORIG, APC], F32, kind="ExternalInput")
    nbr16 = dtn("nbr16", [128, NGRP, EG // 16], I16, kind="ExternalInput")
    self16 = dtn("self16", [128, NGRP, EG // 16], I16, kind="ExternalInput")
    sc16 = dtn("sc16", [128, NGRP, EG // 16], I16, kind="ExternalInput")
    feaT_in = dtn("feaT", [NBR, EGT], BF16, kind="ExternalInput")
    embW = dtn("embW", [ORIG, C], F32, kind="ExternalInput")
    embB = dtn("embB", [1, C], F32, kind="ExternalInput")
    Wn_in = dtn("Wn", [NCONV, C, 2 * C], BF16, kind="ExternalInput")
    Ws_in = dtn("Ws", [NCONV, C, 2 * C], BF16, kind="ExternalInput")
    Wf_in = dtn("Wf", [NCONV, NBR, 2 * C], BF16, kind="ExternalInput")
    bn1g = dtn("bn1g", [NCONV, 128, 1], F32, kind="ExternalInput")
    bn1b = dtn("bn1b", [NCONV, 128, 1], F32, kind="ExternalInput")
    bn2g = dtn("bn2g", [NCONV, 1, C], F32, kind="ExternalInput")
    bn2b = dtn("bn2b", [NCONV, 1, C], F32, kind="ExternalInput")
    headW = dtn("headW", [C, H], F32, kind="ExternalInput")
    headB = dtn("headB", [H, 1], F32, kind="ExternalInput")
    outW = dtn("outW", [H, 1], F32, kind="ExternalInput")
    outBv = dtn("outBv", [1, 1], F32, kind="ExternalInput")
    onehot_in = dtn("onehot", [max(TT, 1), 128, 128], BF16, kind="ExternalInput")
    out = dtn("out", [N0, 1], F32, kind="ExternalOutput")
    if DEBUG_TAPS:
        d_h0 = dtn("d_h0", [APC, C], F32, kind="ExternalOutput")
        d_tbl = dtn("d_tbl", [256, 128], BF16, kind="ExternalOutput")
        d_st1 = dtn("d_st1", [128, 2], F32, kind="ExternalOutput")
        d_spf = dtn("d_spf", [C, 2 * CH], BF16, kind="ExternalOutput")
        d_spc = dtn("d_spc", [C, 2 * CH], BF16, kind="ExternalOutput")
        d_wide = dtn("d_wide", [1536, 128], BF16, kind="ExternalOutput")
        d_sum = dtn("d_sum", [APC, C], F32, kind="ExternalOutput")
        d_st2 = dtn("d_st2", [2, C], F32, kind="ExternalOutput")
        d_h1 = dtn("d_h1", [APC, C], F32, kind="ExternalOutput")
        d_stg = dtn("d_stg", [128, EG // 128, C], BF16, kind="ExternalOutput")
        d_filt = dtn("d_filt", [128, CH // 2], BF16, kind="ExternalOutput")
        d_core = dtn("d_core", [128, CH // 2], BF16, kind="ExternalOutput")

    table = dtn("table", [TBLR, 128], BF16, addr_space="Shared")
    shard_bf = dtn("shard_bf", [SHPAD, 128], BF16)
    h_shard = dtn("h_shard", [APC, C], F32)
    h3_in = dtn("h3_in", [APC, C], BF16)
    h3_full = dtn("h3_full", [N, C], BF16, addr_space="Shared")
    sp_filt = dtn("sp_filt", [C, EGT], BF16)
    sp_core = dtn("sp_core", [C, EGT], BF16)
    wide = dtn("wide", [WIDE_ROWS, 128], BF16)
    st1_in = dtn("st1_in", [128, 2], F32)
    st1_out = dtn("st1_out", [128, 2], F32, addr_space="Shared")
    st1b = dtn("st1b", [128, 2], F32)
    st2_in = dtn("st2_in", [2, C], F32)
    st2_out = dtn("st2_out", [2, C], F32, addr_space="Shared")
    coef2 = dtn("coef2", [2, C], F32)

    def atiles():
        for t in range(NTILE):
            yield t, min(128, APC - t * 128)

    RG = [list(range(R))]

    with tile.TileContext(nc) as tc:
        with tc.tile_pool(name="const", bufs=1) as cp:
            ident = cp.tile([128, 128], BF16)
            make_identity(nc, ident)
            identf = cp.tile([128, 128], F32)
            make_identity(nc, identf)
            zero128 = cp.tile([128, 128], BF16)
            nc.vector.memset(zero128[:], 0.0)
            embW_t = cp.tile([ORIG, C], F32)
            nc.sync.dma_start(out=embW_t[:], in_=embW[:])
            embB_t = cp.tile([128, C], F32)
            nc.gpsimd.dma_start(out=embB_t[:], in_=embB[:].to_broadcast([128, C]))
            eps_t = cp.tile([128, 1], F32)
            nc.vector.memset(eps_t[:], EPS)
            nc.sync.dma_start(out=shard_bf[APC:SHPAD, :], in_=zero128[:SHPAD - APC, :])
            idx_nbr = cp.tile([128, NGRP, EG // 16], I16)
            nc.sync.dma_start(out=idx_nbr[:], in_=nbr16[:])
            idx_self = cp.tile([128, NGRP, EG // 16], I16)
            nc.sync.dma_start(out=idx_self[:], in_=self16[:])
            idx_sc = cp.tile([128, NGRP, EG // 16], I16)
            nc.sync.dma_start(out=idx_sc[:], in_=sc16[:])
            zwide = cp.tile([128, 2048], BF16)
            nc.vector.memset(zwide[:], 0.0)

            # ---------------- embedding ----------------
            with tc.tile_pool(name="emb", bufs=3) as ep, \
                 tc.tile_pool(name="embp", bufs=2, space="PSUM") as epp:
                for t, nrow in atiles():
                    lhs = ep.tile([ORIG, 128], F32, name="lhs")
                    nc.sync.dma_start(out=lhs[:, :nrow], in_=afT[:, ds(t * 128, nrow)])
                    ps = epp.tile([128, C], F32, name="ps")
                    nc.tensor.matmul(ps[:nrow, :], lhsT=lhs[:, :nrow], rhs=embW_t[:])
                    hh = ep.tile([128, C], F32, name="hh")
                    nc.vector.tensor_add(out=hh[:nrow, :], in0=ps[:nrow, :],
                                         in1=embB_t[:nrow, :])
                    nc.sync.dma_start(out=h_shard[ds(t * 128, nrow), :], in_=hh[:nrow, :])
                    stg = ep.tile([128, 128], BF16, name="stg")
                    nc.vector.memset(stg[:], 0.0)
                    nc.vector.tensor_copy(out=stg[:nrow, :C], in_=hh[:nrow, :])
                    nc.sync.dma_start(out=shard_bf[ds(t * 128, nrow), :], in_=stg[:nrow, :])
            nc.gpsimd.collective_compute("AllGather", mybir.AluOpType.bypass,
                                         replica_groups=RG,
                                         ins=[shard_bf[:]], outs=[table[:]])
            if DEBUG_TAPS:
                nc.sync.dma_start(out=d_h0[:], in_=h_shard[:])
                nc.sync.dma_start(out=d_tbl[:], in_=table[0:256, :])

            # ---------------- conv layers ----------------
            for l in range(NCONV):
                last = (l == NCONV - 1)
                wn_t = cp.tile([C, 128], BF16, name=f"wn{l}")
                nc.sync.dma_start(out=wn_t[:], in_=Wn_in[l])
                ws_t = cp.tile([C, 128], BF16, name=f"ws{l}")
                nc.sync.dma_start(out=ws_t[:], in_=Ws_in[l])
                wf_t = cp.tile([NBR, 128], BF16, name=f"wf{l}")
                nc.sync.dma_start(out=wf_t[:], in_=Wf_in[l])

                # zero the scatter accumulator for this layer
                zr = 0
                while zr < NAR * WBS:
                    zn = min(4096, NAR * WBS - zr)
                    nc.sync.dma_start(out=wide[zr:zr + zn, 0:C],
                                      in_=zwide[:, :zn * C // 128])
                    zr += zn

                # ----- pass 1: g matmul + stats + spill -----
                nslots = NGRP * sum(-(-c // 512) for (_, c) in NCH)
                with tc.tile_pool(name="p1g", bufs=2) as p1, \
                     tc.tile_pool(name="p1s", bufs=3) as p1s, \
                     tc.tile_pool(name="p1st", bufs=1) as stp, \
                     tc.tile_pool(name="p1p", bufs=2, space="PSUM") as pp:
                    stats = stp.tile([128, nslots, 6], F32)
                    slot = 0
                    for gi in range(NGRP):
                        w = gi % NWIN
                        gn = p1.tile([128, 1, EG], BF16, name="gn")
                        nc.gpsimd.dma_gather(
                            gn[:], table[w * WROWS:(w + 1) * WROWS, :],
                            idx_nbr[:, gi, :], EG, EG, 128,
                            transpose=True, single_packet=False)
                        gs = p1.tile([128, 1, EG], BF16, name="gs")
                        nc.gpsimd.dma_gather(
                            gs[:], shard_bf[:],
                            idx_self[:, gi, :], EG, EG, 128,
                            transpose=True, single_packet=False)
                        ft = p1.tile([NBR, EG], BF16, name="ft")
                        nc.sync.dma_start(out=ft[:], in_=feaT_in[:, gi * EG:(gi + 1) * EG])
                        for (eo, clen) in NCH:
                            ps = pp.tile([128, CH], F32, name="ps")
                            off = 0
                            while off < clen:
                                wd = min(512, clen - off)
                                sl = ds(eo + off, wd)
                                psl = ds(off, wd)
                                nc.tensor.matmul(ps[:, psl], lhsT=wn_t[:], rhs=gn[:C, 0, sl],
                                                 start=True, stop=False)
                                nc.tensor.matmul(ps[:, psl], lhsT=ws_t[:], rhs=gs[:C, 0, sl],
                                                 start=False, stop=False)
                                nc.tensor.matmul(ps[:, psl], lhsT=wf_t[:], rhs=ft[:, sl],
                                                 start=False, stop=True)
                                nc.vector.bn_stats(out=stats[:, slot, :], in_=ps[:, psl])
                                slot += 1
                                off += wd
                            sp = p1s.tile([128, CH], BF16, name="sp")
                            nc.scalar.activation(out=sp[:, :clen], in_=ps[:, :clen], func=AF.Copy)
                            base = gi * EG + eo
                            nc.sync.dma_start(out=sp_filt[:, base:base + clen], in_=sp[:C, :clen])
                            nc.sync.dma_start(out=sp_core[:, base:base + clen], in_=sp[C:, :clen])
                    agg = stp.tile([128, 2], F32)
                    nc.vector.bn_aggr(out=agg[:], in_=stats[:])
                    sx = stp.tile([128, 2], F32)
                    nc.scalar.mul(out=sx[:, 0:1], in_=agg[:, 0:1], mul=float(EGT))
                    m2 = stp.tile([128, 1], F32)
                    nc.vector.tensor_mul(out=m2[:], in0=agg[:, 0:1], in1=agg[:, 0:1])
                    nc.vector.tensor_add(out=m2[:], in0=m2[:], in1=agg[:, 1:2])
                    nc.scalar.mul(out=sx[:, 1:2], in_=m2[:], mul=float(EGT))
                    nc.sync.dma_start(out=st1_in[:], in_=sx[:])
                nc.gpsimd.collective_compute("AllReduce", mybir.AluOpType.add,
                                             replica_groups=RG,
                                             ins=[st1_in[:]], outs=[st1_out[:]])
                if DEBUG_TAPS and l == 0:
                    nc.sync.dma_start(out=d_st1[:], in_=st1_out[:])
                    nc.sync.dma_start(out=d_spf[:], in_=sp_filt[:, 0:2 * CH])
                    nc.sync.dma_start(out=d_spc[:], in_=sp_core[:, 0:2 * CH])

                with tc.tile_pool(name="coef", bufs=1) as cf:
                    g1 = cf.tile([128, 1], F32)
                    nc.sync.dma_start(out=g1[:], in_=bn1g[l])
                    b1 = cf.tile([128, 1], F32)
                    nc.sync.dma_start(out=b1[:], in_=bn1b[l])
                    stt = cf.tile([128, 2], F32)
                    nc.sync.dma_start(out=stt[:], in_=st1_out[:])
                    ntot = float(N * M)
                    mean = cf.tile([128, 1], F32)
                    nc.scalar.mul(out=mean[:], in_=stt[:, 0:1], mul=1.0 / ntot)
                    var = cf.tile([128, 1], F32)
                    nc.scalar.mul(out=var[:], in_=stt[:, 1:2], mul=1.0 / ntot)
                    msq = cf.tile([128, 1], F32)
                    nc.vector.tensor_mul(out=msq[:], in0=mean[:], in1=mean[:])
                    nc.vector.tensor_sub(out=var[:], in0=var[:], in1=msq[:])
                    sd = cf.tile([128, 1], F32)
                    nc.scalar.activation(out=sd[:], in_=var[:], func=AF.Ln,
                                         bias=eps_t[:], scale=1.0)
                    nc.scalar.activation(out=sd[:], in_=sd[:], func=AF.Exp,
                                         bias=0.0, scale=-0.5)
                    s1 = cf.tile([128, 2], F32)
                    nc.vector.tensor_mul(out=s1[:, 0:1], in0=sd[:], in1=g1[:])
                    nc.vector.tensor_mul(out=s1[:, 1:2], in0=mean[:], in1=s1[:, 0:1])
                    nc.vector.tensor_sub(out=s1[:, 1:2], in0=b1[:], in1=s1[:, 1:2])
                    nc.sync.dma_start(out=st1b[:], in_=s1[:])
                    dup = cf.tile([128, 4], F32)   # [sf tf sc tc] per packed partition
                    nc.sync.dma_start(out=dup[0:64, 0:2], in_=st1b[0:64, :])
                    nc.sync.dma_start(out=dup[64:128, 0:2], in_=st1b[0:64, :])
                    nc.sync.dma_start(out=dup[0:64, 2:4], in_=st1b[64:128, :])
                    nc.sync.dma_start(out=dup[64:128, 2:4], in_=st1b[64:128, :])

                    # ----- pass 2: activations + transpose + scatter -----
                    with tc.tile_pool(name="p2", bufs=3) as p2, \
                         tc.tile_pool(name="p2f", bufs=len(NCH) + 1) as p2f, \
                         tc.tile_pool(name="p2st", bufs=2) as p2s, \
                         tc.tile_pool(name="p2p", bufs=2, space="PSUM") as p2p:
                        for gi in range(NGRP):
                            ar = gi // NWIN
                            stg = p2s.tile([128, EG // 128, C], BF16, name="stg")
                            filts = []
                            for (eo, clen) in NCH:
                                half = clen // 2
                                base = gi * EG + eo
                                filt = p2f.tile([128, CH // 2], BF16, name="filt")
                                nc.sync.dma_start(out=filt[0:C, :half],
                                                  in_=sp_filt[:, base:base + half])
                                nc.sync.dma_start(out=filt[C:128, :half],
                                                  in_=sp_filt[:, base + half:base + clen])
                                nc.scalar.activation(out=filt[:, :half], in_=filt[:, :half],
                                                     func=AF.Sigmoid,
                                                     scale=dup[:, 0:1], bias=dup[:, 1:2])
                                filts.append(filt)
                            for ci, (eo, clen) in enumerate(NCH):
                                half = clen // 2
                                base = gi * EG + eo
                                core = p2.tile([128, CH // 2], BF16, name="core")
                                nc.sync.dma_start(out=core[0:C, :half],
                                                  in_=sp_core[:, base:base + half])
                                nc.sync.dma_start(out=core[C:128, :half],
                                                  in_=sp_core[:, base + half:base + clen])
                                nc.scalar.activation(out=core[:, :half], in_=core[:, :half],
                                                     func=AF.Exp,
                                                     scale=dup[:, 2:3], bias=dup[:, 3:4])
                                nc.scalar.activation(out=core[:, :half], in_=core[:, :half],
                                                     func=AF.Ln, scale=1.0, bias=1.0)
                                if DEBUG_TAPS and l == 0 and gi == 0 and ci == 0:
                                    nc.sync.dma_start(out=d_filt[:], in_=filts[ci][:])
                                    nc.sync.dma_start(out=d_core[:], in_=core[:])
                                msg = p2.tile([128, CH // 2], BF16, name="msg")
                                nc.vector.tensor_mul(out=msg[:, :half], in0=core[:, :half],
                                                     in1=filts[ci][:, :half])
                                for t in range(clen // 256):
                                    tp = p2p.tile([128, 128], BF16, name="tp")
                                    nc.tensor.transpose(tp[:], msg[:, ts(t, 128)], ident[:])
                                    cc = eo // 128 + 2 * t
                                    nc.scalar.activation(
                                        out=stg[:, cc:cc + 2, :].rearrange("p a b -> p (a b)"),
                                        in_=tp[:], func=AF.Copy)
                            if DEBUG_TAPS and l == 0 and gi == 0:
                                nc.sync.dma_start(out=d_stg[:], in_=stg[:])
                            nc.gpsimd.dma_scatter_add(
                                wide[ar * WBS:(ar + 1) * WBS, 0:C],
                                stg[:], idx_sc[:, gi, :], EG, EG, C,
                                elem_step=128, single_packet=False)

                    # ----- reduce over m + bn2 -----
                    with tc.tile_pool(name="rd", bufs=3) as rd, \
                         tc.tile_pool(name="rdp", bufs=1, space="PSUM") as rdp, \
                         tc.tile_pool(name="sm", bufs=1) as smp:
                        ones = smp.tile([128, 1], F32)
                        nc.vector.memset(ones[:], 1.0)
                        sum1 = rdp.tile([1, C], F32)
                        sum2 = rdp.tile([1, C], F32)
                        summed_all = smp.tile([128, NTILE, C], F32)
                        for t, nrow in atiles():
                            loc0 = t * 128
                            arb = loc0 // ARN
                            row0 = arb * WBS + (loc0 - arb * ARN) * M
                            wt = rd.tile([128, M, C], BF16, name="wt")
                            nc.sync.dma_start(
                                out=wt[:nrow],
                                in_=wide[row0:row0 + nrow * M, 0:C]
                                .rearrange("(p w) c -> p w c", w=M))
                            sm = summed_all[:, t, :]
                            nc.vector.reduce_sum(
                                out=sm[:nrow].rearrange("p (c u) -> p c u", u=1),
                                in_=wt[:nrow].rearrange("p w c -> p c w"),
                                axis=mybir.AxisListType.X)
                            nc.tensor.matmul(sum1[:], lhsT=ones[:nrow], rhs=sm[:nrow],
                                             start=(t == 0), stop=(t == NTILE - 1))
                            sq = rd.tile([128, C], F32, name="sq")
                            nc.vector.tensor_mul(out=sq[:nrow], in0=sm[:nrow], in1=sm[:nrow])
                            nc.tensor.matmul(sum2[:], lhsT=ones[:nrow], rhs=sq[:nrow],
                                             start=(t == 0), stop=(t == NTILE - 1))
                        if DEBUG_TAPS and l == 0:
                            nc.sync.dma_start(out=d_wide[:], in_=wide[0:1536, :])
                            for t, nrow in atiles():
                                nc.sync.dma_start(out=d_sum[ds(t * 128, nrow), :],
                                                  in_=summed_all[:nrow, t, :])
                        s2sb = smp.tile([1, 2, C], F32)
                        nc.scalar.activation(out=s2sb[:, 0, :], in_=sum1[:], func=AF.Copy)
                        nc.scalar.activation(out=s2sb[:, 1, :], in_=sum2[:], func=AF.Copy)
                        nc.sync.dma_start(out=st2_in[0:1, :], in_=s2sb[:, 0, :])
                        nc.sync.dma_start(out=st2_in[1:2, :], in_=s2sb[:, 1, :])
                        nc.gpsimd.collective_compute("AllReduce", mybir.AluOpType.add,
                                                     replica_groups=RG,
                                                     ins=[st2_in[:]], outs=[st2_out[:]])
                        if DEBUG_TAPS and l == 0:
                            nc.sync.dma_start(out=d_st2[:], in_=st2_out[:])
                        g2 = smp.tile([1, C], F32)
                        nc.sync.dma_start(out=g2[:], in_=bn2g[l])
                        b2 = smp.tile([1, C], F32)
                        nc.sync.dma_start(out=b2[:], in_=bn2b[l])
                        st2 = smp.tile([1, 2, C], F32)
                        nc.sync.dma_start(out=st2[:], in_=st2_out[:].rearrange("(u a) c -> u a c", u=1))
                        mean2 = smp.tile([1, C], F32)
                        nc.scalar.mul(out=mean2[:], in_=st2[:, 0, :], mul=1.0 / N)
                        var2 = smp.tile([1, C], F32)
                        nc.scalar.mul(out=var2[:], in_=st2[:, 1, :], mul=1.0 / N)
                        m2q = smp.tile([1, C], F32)
                        nc.vector.tensor_mul(out=m2q[:], in0=mean2[:], in1=mean2[:])
                        nc.vector.tensor_sub(out=var2[:], in0=var2[:], in1=m2q[:])
                        sd2 = smp.tile([1, C], F32)
                        nc.scalar.activation(out=sd2[:], in_=var2[:], func=AF.Ln,
                                             bias=eps_t[0:1, :], scale=1.0)
                        nc.scalar.activation(out=sd2[:], in_=sd2[:], func=AF.Exp,
                                             bias=0.0, scale=-0.5)
                        s2c = smp.tile([1, C], F32)
                        nc.vector.tensor_mul(out=s2c[:], in0=sd2[:], in1=g2[:])
                        t2c = smp.tile([1, C], F32)
                        nc.vector.tensor_mul(out=t2c[:], in0=mean2[:], in1=s2c[:])
                        nc.vector.tensor_sub(out=t2c[:], in0=b2[:], in1=t2c[:])
                        nc.sync.dma_start(out=coef2[0:1, :], in_=s2c[:])
                        nc.sync.dma_start(out=coef2[1:2, :], in_=t2c[:])
                        s2r = smp.tile([128, C], F32)
                        nc.gpsimd.dma_start(out=s2r[:], in_=coef2[0:1, :].to_broadcast([128, C]))
                        t2r = smp.tile([128, C], F32)
                        nc.gpsimd.dma_start(out=t2r[:], in_=coef2[1:2, :].to_broadcast([128, C]))

                        # ----- finalize h_new -----
                        with tc.tile_pool(name="fin", bufs=3) as fp:
                            for t, nrow in atiles():
                                sm = summed_all[:, t, :]
                                x = fp.tile([128, C], F32, name="x")
                                nc.vector.tensor_mul(out=x[:nrow], in0=sm[:nrow],
                                                     in1=s2r[:nrow, :])
                                nc.vector.tensor_add(out=x[:nrow], in0=x[:nrow],
                                                     in1=t2r[:nrow, :])
                                ho = fp.tile([128, C], F32, name="ho")
                                nc.sync.dma_start(out=ho[:nrow],
                                                  in_=h_shard[ds(t * 128, nrow), :])
                                nc.vector.tensor_add(out=x[:nrow], in0=x[:nrow], in1=ho[:nrow])
                                hn = fp.tile([128, C], F32, name="hn")
                                nc.scalar.activation(out=hn[:nrow], in_=x[:nrow], func=AF.Exp)
                                nc.scalar.activation(out=hn[:nrow], in_=hn[:nrow],
                                                     func=AF.Ln, scale=1.0, bias=1.0)
                                if not last:
                                    nc.sync.dma_start(out=h_shard[ds(t * 128, nrow), :],
                                                      in_=hn[:nrow])
                                stg2 = fp.tile([128, 128], BF16, name="stg2")
                                nc.vector.memset(stg2[:], 0.0)
                                nc.vector.tensor_copy(out=stg2[:nrow, :C], in_=hn[:nrow])
                                if last:
                                    nc.sync.dma_start(out=h3_in[ds(t * 128, nrow), :],
                                                      in_=stg2[:nrow, :C])
                                else:
                                    nc.sync.dma_start(out=shard_bf[ds(t * 128, nrow), :],
                                                      in_=stg2[:nrow, :])
                if DEBUG_TAPS and l == 0:
                    nc.sync.dma_start(out=d_h1[:], in_=h_shard[:])
                if not last:
                    nc.gpsimd.collective_compute("AllGather", mybir.AluOpType.bypass,
                                                 replica_groups=RG,
                                                 ins=[shard_bf[:]], outs=[table[:]])

            # ---------------- pooling + head (redundant on all cores) -------
            nc.gpsimd.collective_compute("AllGather", mybir.AluOpType.bypass,
                                         replica_groups=RG,
                                         ins=[h3_in[:]], outs=[h3_full[:]])
            with tc.tile_pool(name="pl", bufs=3) as pl, \
                 tc.tile_pool(name="plc", bufs=1) as plc, \
                 tc.tile_pool(name="plp", bufs=2, space="PSUM") as plp:
                hw_t = plc.tile([C, H], F32)
                nc.sync.dma_start(out=hw_t[:], in_=headW[:])
                hb_t = plc.tile([H, 1], F32)
                nc.sync.dma_start(out=hb_t[:], in_=headB[:])
                ow_t = plc.tile([H, 1], F32)
                nc.sync.dma_start(out=ow_t[:], in_=outW[:])
                ob_t = plc.tile([1, 1], F32)
                nc.sync.dma_start(out=ob_t[:], in_=outBv[:])
                ti = 0
                for (bc0, bc1, t0, t1) in bands:
                    nct = bc1 - bc0
                    crys_ps = plp.tile([128, C], F32, name="crys_ps")
                    if t1 == t0:
                        nc.vector.memset(crys_ps[:], 0.0)
                    for k, t in enumerate(range(t0, t1)):
                        nrow2 = min(128, N - t * 128)
                        oh = pl.tile([128, 128], BF16, name="oh")
                        nc.sync.dma_start(out=oh[:], in_=onehot_in[ti])
                        hrow = pl.tile([128, C], BF16, name="hrow")
                        if nrow2 < 128:
                            nc.vector.memset(hrow[:], 0.0)
                        nc.sync.dma_start(out=hrow[:nrow2],
                                          in_=h3_full[ds(t * 128, nrow2), :])
                        nc.tensor.matmul(crys_ps[:], lhsT=oh[:], rhs=hrow[:],
                                         start=(k == 0), stop=(k == t1 - t0 - 1))
                        ti += 1
                    spc = pl.tile([128, C], F32, name="spc")
                    nc.scalar.activation(out=spc[:], in_=crys_ps[:], func=AF.Exp)
                    nc.scalar.activation(out=spc[:], in_=spc[:], func=AF.Ln,
                                         scale=1.0, bias=1.0)
                    spcT_ps = plp.tile([C, 128], F32, name="spcT_ps")
                    nc.tensor.transpose(spcT_ps[:], spc[:], identf[:])
                    spcT = pl.tile([C, 128], F32, name="spcT")
                    nc.scalar.activation(out=spcT[:], in_=spcT_ps[:], func=AF.Copy)
                    z_ps = plp.tile([H, 128], F32, name="z_ps")
                    nc.tensor.matmul(z_ps[:], lhsT=hw_t[:], rhs=spcT[:])
                    z = pl.tile([H, 128], F32, name="z")
                    nc.scalar.activation(out=z[:], in_=z_ps[:], func=AF.Exp,
                                         bias=hb_t[:], scale=1.0)
                    nc.scalar.activation(out=z[:], in_=z[:], func=AF.Ln,
                                         scale=1.0, bias=1.0)
                    o_ps = plp.tile([1, 128], F32, name="o_ps")
                    nc.tensor.matmul(o_ps[:], lhsT=ow_t[:], rhs=z[:])
                    ov = pl.tile([1, 128], F32, name="ov")
                    nc.scalar.activation(out=ov[:], in_=o_ps[:], func=AF.Identity,
                                         bias=ob_t[0:1, :], scale=1.0)
                    nc.sync.dma_start(out=out[bc0:bc1, :], in_=ov[:, :nct])
    nc.compile()
    return nc


def make_inputs(inputs, cores, EG, EGT, TT, onehot):
    fc_W = np.asarray(inputs["fc_W"], np.float32)        # [3, 169, 128]
    base = dict(
        embW=np.asarray(inputs["emb_W"], np.float32),
        embB=np.asarray(inputs["emb_b"], np.float32).reshape(1, C),
        Wn=fc_W[:, C:2 * C, :].astype(BF),
        Ws=fc_W[:, :C, :].astype(BF),
        Wf=fc_W[:, 2 * C:, :].astype(BF),
        bn1g=np.asarray(inputs["bn1_g"], np.float32).reshape(NCONV, 128, 1),
        bn1b=np.asarray(inputs["bn1_b"], np.float32).reshape(NCONV, 128, 1),
        bn2g=np.asarray(inputs["bn2_g"], np.float32).reshape(NCONV, 1, C),
        bn2b=np.asarray(inputs["bn2_b"], np.float32).reshape(NCONV, 1, C),
        headW=np.asarray(inputs["head_W"], np.float32),
        headB=np.asarray(inputs["head_b"], np.float32).reshape(H, 1),
        outW=np.asarray(inputs["out_W"], np.float32).reshape(H, 1),
        outBv=np.asarray(inputs["out_b"], np.float32).reshape(1, 1),
        onehot=onehot,
    )
    af = np.asarray(inputs["atom_fea"], np.float32)
    APC = N // R
    in_maps = []
    for r in range(R):
        m = dict(base)
        m["afT"] = np.ascontiguousarray(af[r * APC:(r + 1) * APC].T)
        m.update(cores[r])
        in_maps.append(m)
    return in_maps


_BUILT = {}


def kernel(**inputs):
    from concourse.bass_utils import run_bass_kernel_spmd
    cores, EG, EGT, bands, TT, onehot = prep_host(
        inputs["nbr_fea_idx"], inputs["nbr_fea"], inputs["crystal_atom_idx"])
    key = (EG, TT, tuple(b for b in map(tuple, bands)))
    if key not in _BUILT:
        _BUILT[key] = build(EG, bands, TT)
    nc = _BUILT[key]
    in_maps = make_inputs(inputs, cores, EG, EGT, TT, onehot)
    res = run_bass_kernel_spmd(nc, in_maps, list(range(R)))
    return res.results[0]["out"].astype(np.float32)

